# revision 61
# baseline (speedup 1.0000x reference)
"""Deformable conv (nn_DeformConv) Trainium2 Bass kernel. V5.

Per core = one batch element (data-parallel over 8 cores).

Pipeline:
  1. x -> xbf (bf16, zero-padded 72x72 grid); x ct0 via SP DMA, ct1 via Act.
  2. 1x1 conv on PE (bf16) -> x_chan; depthwise 3x3 on PE via per-tap diagonal
     weight matmuls accumulated in PSUM (bias fused in the Act evac).
  3. PE-transpose offsets to position-partition layout; batched floor/residual
     math -> bilinear weights wts_sb and gather row index r0_sb (int32),
     clamped per tile to the table prefix that tile can reach.
  4. DRAM table [5248, 1024] bf16, row r = [x | Dx | Dy | Dxy](r): chunked
     diffs (DVE/Pool), PE transposes, DVE/Pool evacs, SP DMAs.  Main-loop
     gathers reference only their table prefix, and all main-loop SBUF/PSUM
     pools are pre-allocated disjoint from the build pools, so the main loop
     overlaps the tail of the table build.
  5. w_defT loaded pre-transposed by one strided DMA (no PE work).
  6. Main loop per 128-position tile: 9 per-tap indirect row gathers (Pool,
     issued LOOKAHEAD tiles ahead); bilinear combine fused into the PE
     transpose as PSUM-accumulated diag matmuls (diags = ident * weight
     broadcast, built on DVE); 36-matmul PSUM contraction vs w_def; per-tile
     output DMA.
"""
import os
import numpy as np
from contextlib import ExitStack

import concourse.bass as bass
import concourse.mybir as mybir
import concourse.tile as tile
from concourse import bacc as _bacc
from concourse.bass import IndirectOffsetOnAxis
from concourse.masks import make_identity

FP32 = mybir.dt.float32
BF16 = mybir.dt.bfloat16
I32 = mybir.dt.int32

N, C, H, W = 8, 256, 64, 64
HW = H * W                    # 4096
K = 9
OFFC = 18
PAD = 4
G = H + 2 * PAD               # 72
ROWS = G * G                  # 5184
RT = 5248                     # rows padded to 41*128
NRT = RT // 128               # 41
NPT = HW // 128               # 32 position tiles
CT = C // 128                 # 2 channel tiles
KT = (C * K) // 128           # 18 contraction tiles
RMAX = RT - G - 2             # global row clamp
ALU = mybir.AluOpType
AF = mybir.ActivationFunctionType

MODES = os.environ.get("MODES", "ppppppppp")     # per-tap combine mode (d/a/p)
EVACS = os.environ.get("EVACS", "ADADA")         # sampT evac engines (5 groups)
OUT_EVAC = os.environ.get("OUT_EVAC", "A")       # out evac engine
GBUFS = int(os.environ.get("GBUFS", "3"))
DGBUFS = int(os.environ.get("DGBUFS", "1"))
LOOKAHEAD = int(os.environ.get("LOOKAHEAD", "2"))
DIFF_CHUNKS = int(os.environ.get("DIFF_CHUNKS", "4"))


def tile_row_bound(t):
    # max table row tile t can touch, assuming |offset| <= 4 (measured 0.81)
    return min(RMAX, 144 * t + 791)


def build_nc():
    nc = _bacc.Bacc()
    x_d = nc.dram_tensor("x", [C, HW], FP32, kind="ExternalInput")
    w_adj_d = nc.dram_tensor("w_adj", [OFFC, C], FP32, kind="ExternalInput")
    b_adj_d = nc.dram_tensor("b_adj", [OFFC, 1], FP32, kind="ExternalInput")
    w_off_d = nc.dram_tensor("w_off", [OFFC, K], FP32, kind="ExternalInput")
    b_off_d = nc.dram_tensor("b_off", [OFFC, 1], FP32, kind="ExternalInput")
    w_def_d = nc.dram_tensor("w_def", [C, C * K], FP32, kind="ExternalInput")
    out_d = nc.dram_tensor("out", [C, HW], FP32, kind="ExternalOutput")

    any_da = any(m in "da" for m in MODES)

    with tile.TileContext(nc) as tc, ExitStack() as ctx:
        pers = ctx.enter_context(tc.tile_pool(name="pers", bufs=1))
        dram = ctx.enter_context(tc.tile_pool(name="dram", bufs=1, space="DRAM"))
        # main-loop pools pre-allocated so their SBUF/PSUM never overlaps the
        # table-build pools (otherwise WAR hazards stall the loop start)
        gat = ctx.enter_context(tc.tile_pool(name="gat", bufs=GBUFS))
        smp = ctx.enter_context(tc.tile_pool(name="smp", bufs=2))
        dgp = ctx.enter_context(tc.tile_pool(name="dgp", bufs=DGBUFS))
        oev = ctx.enter_context(tc.tile_pool(name="oev", bufs=3))
        psS = ctx.enter_context(tc.tile_pool(name="psS", bufs=3, space="PSUM"))
        psO = ctx.enter_context(tc.tile_pool(name="psO", bufs=2, space="PSUM"))
        psB = ctx.enter_context(tc.tile_pool(name="psB", bufs=1, space="PSUM"))

        table = dram.tile([RT, 4 * C], BF16)

        ident_f = pers.tile([128, 128], FP32)
        make_identity(nc, ident_f[:])
        ident_b = pers.tile([128, 128], BF16)
        nc.vector.tensor_copy(ident_b[:], ident_f[:])
        ident3 = pers.tile([128, 3, 128], BF16)
        for s in range(3):
            nc.vector.tensor_copy(ident3[:, s, :], ident_b[:])

        # per-partition constants: hh = p//64 (0/1), ww = p%64
        iota_p = pers.tile([128, 1], I32)
        nc.gpsimd.iota(iota_p[:], pattern=[[0, 1]], base=0, channel_multiplier=1)
        pf = pers.tile([128, 1], FP32)
        nc.vector.tensor_copy(pf[:], iota_p[:])
        hh = pers.tile([128, 1], FP32)
        nc.vector.tensor_scalar(out=hh[:], in0=pf[:], scalar1=64.0, scalar2=None,
                                op0=ALU.is_ge)
        ww = pers.tile([128, 1], FP32)
        nc.vector.scalar_tensor_tensor(out=ww[:], in0=hh[:], scalar=-64.0,
                                       in1=pf[:], op0=ALU.mult, op1=ALU.add)

        by_f = pers.tile([128, NPT, K], FP32)
        bx_f = pers.tile([128, NPT, K], FP32)
        bnd_f = pers.tile([128, NPT, K], FP32)

        w_defT = pers.tile([128, KT, 2 * 128], BF16)   # [ck-part, kt, o]
        r0_sb = pers.tile([128, NPT, K], I32)
        wts_sb = pers.tile([128, NPT, K * 3], FP32)    # k-major (rx, ry, rxry)

        # ---------------- x load (SP + Act in parallel) + xbf ----------------
        xs_stack = ExitStack()
        xp = xs_stack.enter_context(tc.tile_pool(name="xp", bufs=1))
        xbf = xp.tile([128, CT, RT], BF16)
        xl = ExitStack()
        xlp = xl.enter_context(tc.tile_pool(name="xl", bufs=1))
        x_sb = xlp.tile([128, CT, HW], FP32)
        nc.sync.dma_start(out=x_sb[:, 0, :], in_=x_d[0:128, :])
        nc.scalar.dma_start(out=x_sb[:, 1, :], in_=x_d[128:256, :])

        # batched base ramps over (t, k): by = 2t + ki + (PAD-1), bx = kj + (PAD-1)
        # (int staging tiles live in this transient pool)
        iti = xlp.tile([128, 3, NPT, K], I32)
        nc.gpsimd.iota(iti[:, 0], pattern=[[2, NPT], [1, 3], [0, 3]], base=PAD - 1,
                       channel_multiplier=0)
        nc.gpsimd.iota(iti[:, 1], pattern=[[0, NPT], [0, 3], [1, 3]], base=PAD - 1,
                       channel_multiplier=0)
        nc.gpsimd.iota(iti[:, 2], pattern=[[144, NPT], [0, K]], base=791,
                       channel_multiplier=0)
        nc.vector.tensor_copy(by_f[:], iti[:, 0])
        nc.vector.tensor_copy(bx_f[:], iti[:, 1])
        nc.vector.tensor_copy(bnd_f[:], iti[:, 2])
        nc.vector.tensor_scalar(out=bnd_f[:], in0=bnd_f[:], scalar1=float(RMAX),
                                scalar2=None, op0=ALU.min)
        for ct in range(CT):
            xv = xbf[:, ct, :ROWS].rearrange("p (h w) -> p h w", h=G, w=G)
            eng = nc.vector if ct == 0 else nc.gpsimd
            eng.memset(xbf[:, ct, :PAD * G], 0.0)
            eng.memset(xbf[:, ct, (PAD + H) * G:], 0.0)
            eng.memset(xv[:, PAD:PAD + H, 0:PAD], 0.0)
            eng.memset(xv[:, PAD:PAD + H, PAD + W:G], 0.0)
            eng.tensor_copy(
                xv[:, PAD:PAD + H, PAD:PAD + W],
                x_sb[:, ct, :].rearrange("p (h w) -> p h w", h=H, w=W))
        xl.close()

        # table pool opened before offp so pool closes stay LIFO
        tbl_stack = ExitStack()
        tblp = tbl_stack.enter_context(tc.tile_pool(name="tblp", bufs=1))
        dbf = tblp.tile([128, CT, 3, RT], BF16)
        evb = tbl_stack.enter_context(tc.tile_pool(name="evb", bufs=3))
        EARLY_RT = int(os.environ.get("EARLY_RT", "8"))

        def emit_table_rt(rt, late=False):
            tb = evb.tile([128, 4, C], BF16, tag="tb")
            for ct in range(CT):
                ps = psB.tile([128, 4 * 128], BF16, tag="ps")
                nc.tensor.transpose(ps[:, 0:128],
                                    xbf[:, ct, rt * 128:(rt + 1) * 128], ident_b[:])
                for s in range(3):
                    nc.tensor.transpose(
                        ps[:, (s + 1) * 128:(s + 2) * 128],
                        dbf[:, ct, s, rt * 128:(rt + 1) * 128], ident_b[:])
                tbv = tb[:, :, ct * 128:(ct + 1) * 128]
                psv = ps[:].rearrange("p (s c) -> p s c", s=4)
                if late or ct == 1:
                    nc.scalar.copy(tbv, psv)
                else:
                    nc.vector.tensor_copy(tbv, psv)
            nc.sync.dma_start(out=table[rt * 128:(rt + 1) * 128, :], in_=tb[:])

        # ---------------- phase 1: offsets pipeline ----------------
        ph1 = ExitStack()
        offp = ph1.enter_context(tc.tile_pool(name="offp", bufs=1))
        convs = ExitStack()
        psA = convs.enter_context(tc.tile_pool(name="psA", bufs=2, space="PSUM"))
        psD = psA

        w_adjT = offp.tile([128, CT, OFFC], FP32)
        for ct in range(CT):
            nc.sync.dma_start(
                out=w_adjT[:, ct, :],
                in_=w_adj_d.rearrange("o c -> c o")[ct * 128:(ct + 1) * 128, :])
        w_adjT_b = offp.tile([128, CT, OFFC], BF16)
        nc.vector.tensor_copy(w_adjT_b[:], w_adjT[:])
        b_adj_sb = offp.tile([OFFC, 1], FP32)
        nc.sync.dma_start(out=b_adj_sb[:], in_=b_adj_d[:, :])
        w_off_sb = offp.tile([OFFC, K], FP32)
        nc.sync.dma_start(out=w_off_sb[:], in_=w_off_d[:, :])
        b_off_sb = offp.tile([OFFC, 1], FP32)
        nc.sync.dma_start(out=b_off_sb[:], in_=b_off_d[:, :])
        w_off_b = offp.tile([OFFC, K, 1], BF16)
        nc.vector.tensor_copy(w_off_b[:], w_off_sb[:].rearrange("p (k o) -> p k o", o=1))

        # w_defT via strided transposing DMAs (SP, after the small loads):
        # w_defT[c%128, (k, chalf), o] = w_def[o, (c k)]
        with tc.tile_pool(name="wdefp", bufs=1) as wdefp:
            wd_view = w_def_d.rearrange("o (ch cc k) -> cc k ch o",
                                        ch=CT, cc=128, k=K)
            for k in range(K):
                stage = wdefp.tile([128, CT, 2 * 128], FP32, tag="wds")
                for ch in range(CT):
                    nc.sync.dma_start(out=stage[:, ch, :], in_=wd_view[:, k, ch, :])
                nc.vector.tensor_copy(
                    w_defT[:, 2 * k:2 * k + 2, :].rearrange(
                        "p kt o -> p (kt o)"),
                    stage[:].rearrange("p c o -> p (c o)"))

        # depthwise diag weights: dwdiag[p, tap, j] = (j==p) ? w_off[p,tap] : 0
        dwdiag = offp.tile([OFFC, K, OFFC], BF16)
        nc.gpsimd.affine_select(
            out=dwdiag[:],
            in_=w_off_b[:].broadcast_to([OFFC, K, OFFC]),
            pattern=[[0, K], [1, OFFC]],
            compare_op=ALU.is_equal,
            fill=0.0,
            base=0,
            channel_multiplier=-1)

        # ---------------- table diffs (early: right after xbf) ----------------
        NCH = DIFF_CHUNKS
        bounds = [(ch * RT // NCH, (ch + 1) * RT // NCH) for ch in range(NCH)]

        def emit_dxy(ch, ct, deng):
            lo, hi = bounds[ch]
            h2 = min(hi, RT - G)
            if lo < h2:
                deng.tensor_tensor(out=dbf[:, ct, 2, lo:h2],
                                   in0=dbf[:, ct, 0, lo + G:h2 + G],
                                   in1=dbf[:, ct, 0, lo:h2], op=ALU.subtract)

        for ch in range(NCH):
            lo, hi = bounds[ch]
            for ct in range(CT):
                deng = nc.vector if (ch + ct) % 2 == 0 else nc.gpsimd
                h1 = min(hi, RT - 1)
                deng.tensor_tensor(out=dbf[:, ct, 0, lo:h1],
                                   in0=xbf[:, ct, lo + 1:h1 + 1],
                                   in1=xbf[:, ct, lo:h1], op=ALU.subtract)
                h2 = min(hi, RT - G)
                if lo < h2:
                    deng.tensor_tensor(out=dbf[:, ct, 1, lo:h2],
                                       in0=xbf[:, ct, lo + G:h2 + G],
                                       in1=xbf[:, ct, lo:h2], op=ALU.subtract)
                if ch >= 1:
                    emit_dxy(ch - 1, ct, deng)
        for ct in range(CT):
            nc.gpsimd.memset(dbf[:, ct, 0, RT - 1:RT], 0.0)
            emit_dxy(NCH - 1, ct, nc.vector if ct == 0 else nc.gpsimd)
            nc.gpsimd.memset(dbf[:, ct, 1, RT - G:RT], 0.0)
            nc.gpsimd.memset(dbf[:, ct, 2, RT - G:RT], 0.0)

        # ---------------- 1x1 conv + depthwise (PE/Act) ----------------
        GC = H + 2   # 66
        xch_pad = offp.tile([OFFC, GC * GC], BF16)
        xch_v = xch_pad[:].rearrange("p (h w) -> p h w", h=GC, w=GC)
        nc.gpsimd.memset(xch_pad[:, 0:GC], 0.0)
        nc.gpsimd.memset(xch_pad[:, (GC - 1) * GC:], 0.0)
        nc.gpsimd.memset(xch_v[:, 1:GC - 1, 0:1], 0.0)
        nc.gpsimd.memset(xch_v[:, 1:GC - 1, GC - 1:GC], 0.0)
        xin_v = xbf[:, :, :ROWS].rearrange("p c (h w) -> p c h w", h=G, w=G)
        for pch in range(8):
            ps = psA.tile([OFFC, 512], FP32, tag="cdw")
            for ct in range(CT):
                rhs = xin_v[:, ct, PAD + pch * 8:PAD + pch * 8 + 8, PAD:PAD + W]
                nc.tensor.matmul(out=ps[:], lhsT=w_adjT_b[:, ct, :],
                                 rhs=rhs,
                                 start=(ct == 0), stop=(ct == CT - 1))
            nc.scalar.activation(
                out=xch_v[:, 1 + pch * 8:1 + pch * 8 + 8, 1:1 + W],
                in_=ps[:].rearrange("p (h w) -> p h w", h=8, w=W),
                func=AF.Identity, bias=b_adj_sb[:], scale=1.0)

        off_sb = offp.tile([OFFC, HW], BF16)
        scr_stack = ExitStack()
        scr = scr_stack.enter_context(tc.tile_pool(name="scr", bufs=1))
        offT = scr.tile([128, NPT, OFFC], FP32)

        for c8 in range(8):
            psd = psD.tile([OFFC, 512], FP32, tag="cdw")
            for tap in range(K):
                di, dj = tap // 3, tap % 3
                rhs = xch_v[:, di + 8 * c8:di + 8 * c8 + 8, dj:dj + W]
                nc.tensor.matmul(out=psd[:], lhsT=dwdiag[:, tap, :], rhs=rhs,
                                 start=(tap == 0), stop=(tap == K - 1))
            nc.scalar.activation(
                out=off_sb[:, c8 * 512:(c8 + 1) * 512],
                in_=psd[:], func=AF.Identity, bias=b_off_sb[:], scale=1.0)
        convs.close()

        for rt in range(EARLY_RT):
            emit_table_rt(rt)

        with tc.tile_pool(name="psT", bufs=2, space="PSUM") as psT:
            for tg in range(NPT // 4):   # 4 transposes per psum tile, one evac
                pso = psT.tile([128, 4, OFFC], BF16, tag="pst")
                for j in range(4):
                    t = tg * 4 + j
                    nc.tensor.transpose(pso[:, j, :],
                                        off_sb[:, t * 128:(t + 1) * 128],
                                        ident_b[:OFFC, :OFFC])
                if tg % 2 == 0:
                    nc.scalar.copy(offT[:, tg * 4:tg * 4 + 4, :], pso[:])
                else:
                    nc.vector.tensor_copy(offT[:, tg * 4:tg * 4 + 4, :], pso[:])

        # ------------- index math (two t-halves; first half unblocks the
        # early gathers sooner) -------------
        py = scr.tile([128, NPT, K], FP32)
        px = scr.tile([128, NPT, K], FP32)
        fyi = scr.tile([128, NPT, K], I32)
        fy = scr.tile([128, NPT, K], FP32)
        fx = scr.tile([128, NPT, K], FP32)
        m = scr.tile([128, NPT, K], FP32)
        r0f = scr.tile([128, NPT, K], FP32)
        for lo, hi in ((0, NPT // 2), (NPT // 2, NPT)):
            sl = slice(lo, hi)
            dyv = offT[:, sl].rearrange("p t (k two) -> p t k two", two=2)[:, :, :, 0]
            dxv = offT[:, sl].rearrange("p t (k two) -> p t k two", two=2)[:, :, :, 1]
            nc.vector.scalar_tensor_tensor(out=py[:, sl], in0=dyv, scalar=hh[:, 0:1],
                                           in1=by_f[:, sl], op0=ALU.add, op1=ALU.add)
            nc.vector.scalar_tensor_tensor(out=px[:, sl], in0=dxv, scalar=ww[:, 0:1],
                                           in1=bx_f[:, sl], op0=ALU.add, op1=ALU.add)
            nc.vector.tensor_copy(fyi[:, sl], py[:, sl])
            nc.vector.tensor_copy(fy[:, sl], fyi[:, sl])
            nc.vector.tensor_tensor(out=m[:, sl], in0=fy[:, sl], in1=py[:, sl],
                                    op=ALU.is_gt)
            nc.vector.tensor_sub(out=fy[:, sl], in0=fy[:, sl], in1=m[:, sl])
            nc.vector.tensor_copy(fyi[:, sl], px[:, sl])
            nc.vector.tensor_copy(fx[:, sl], fyi[:, sl])
            nc.vector.tensor_tensor(out=m[:, sl], in0=fx[:, sl], in1=px[:, sl],
                                    op=ALU.is_gt)
            nc.vector.tensor_sub(out=fx[:, sl], in0=fx[:, sl], in1=m[:, sl])
            # residuals, k-major slots (rx, ry, rxry)
            wv = wts_sb[:].rearrange("p t (k s) -> p t k s", s=3)[:, sl]
            nc.vector.tensor_sub(out=wv[:, :, :, 0], in0=px[:, sl], in1=fx[:, sl])
            nc.vector.tensor_sub(out=wv[:, :, :, 1], in0=py[:, sl], in1=fy[:, sl])
            nc.vector.tensor_tensor(out=wv[:, :, :, 2], in0=wv[:, :, :, 0],
                                    in1=wv[:, :, :, 1], op=ALU.mult)
            nc.vector.scalar_tensor_tensor(out=r0f[:, sl], in0=fy[:, sl],
                                           scalar=float(G), in1=fx[:, sl],
                                           op0=ALU.mult, op1=ALU.add)
            nc.vector.tensor_scalar(out=r0f[:, sl], in0=r0f[:, sl], scalar1=0.0,
                                    scalar2=None, op0=ALU.max)
            nc.vector.tensor_tensor(out=r0f[:, sl], in0=r0f[:, sl],
                                    in1=bnd_f[:, sl], op=ALU.min)
            nc.vector.tensor_copy(r0_sb[:, sl], r0f[:, sl])
        scr_stack.close()
        ph1.close()

        for rt in range(EARLY_RT, NRT):
            emit_table_rt(rt, late=True)
        tbl_stack.close()
        xs_stack.close()

        # ---------------- phase 4: main loop ----------------
        assert len(MODES) == K and set(MODES) <= set("dap")
        assert len(EVACS) == 5 and set(EVACS) <= set("ADP")
        gq = {}
        dq = {}

        def issue_d3(t):
            d3n = dgp.tile([128, K, 3, 128], BF16, tag="d3")
            for k in range(K):
                if MODES[k] == "p":
                    nc.vector.tensor_tensor(
                        out=d3n[:, k, :, :], in0=ident3[:],
                        in1=wts_sb[:, t, 3 * k:3 * k + 3]
                            .rearrange("p (s o) -> p s o", o=1)
                            .broadcast_to([128, 3, 128]),
                        op=ALU.mult)
            dq[t] = d3n

        def issue_gather(t):
            g = gat.tile([128, K, 4 * C], BF16, tag="g")
            hi_row = tile_row_bound(t) + 1
            hi_rt = min(NRT, (hi_row + 127) // 128)
            for k in range(K):
                nc.gpsimd.indirect_dma_start(
                    out=g[:, k, :], out_offset=None,
                    in_=table[0:hi_rt * 128, :],
                    in_offset=IndirectOffsetOnAxis(ap=r0_sb[:, t, k:k + 1], axis=0))
            gq[t] = g

        DLOOK = min(int(os.environ.get("DLOOK", "1")), DGBUFS - 1) if DGBUFS > 1 else 0
        for t in range(min(LOOKAHEAD, NPT)):
            issue_gather(t)
        for t in range(min(DLOOK + 1, NPT)):
            issue_d3(t)
        for t in range(NPT):
            if t + LOOKAHEAD < NPT:
                issue_gather(t + LOOKAHEAD)
            if t + DLOOK + 1 < NPT and DLOOK > 0:
                issue_d3(t + DLOOK + 1)
            elif DLOOK == 0 and t > 0:
                issue_d3(t)
            g_sb = gq.pop(t)
            if any_da:
                samp = smp.tile([128, KT * 128], BF16, tag="s")
                atmp = smp.tile([128, 2, C], BF16, tag="at")
            d3 = dq.pop(t)
            for k in range(K):
                mode = MODES[k]
                if mode != "p":
                    av = samp[:, k * C:(k + 1) * C]
                if mode == "d":
                    nc.vector.scalar_tensor_tensor(
                        out=av, in0=g_sb[:, k, C:2 * C],
                        scalar=wts_sb[:, t, 3 * k:3 * k + 1],
                        in1=g_sb[:, k, 0:C], op0=ALU.mult, op1=ALU.add)
                    nc.vector.scalar_tensor_tensor(
                        out=av, in0=g_sb[:, k, 2 * C:3 * C],
                        scalar=wts_sb[:, t, 3 * k + 1:3 * k + 2],
                        in1=av, op0=ALU.mult, op1=ALU.add)
                    nc.vector.scalar_tensor_tensor(
                        out=av, in0=g_sb[:, k, 3 * C:4 * C],
                        scalar=wts_sb[:, t, 3 * k + 2:3 * k + 3],
                        in1=av, op0=ALU.mult, op1=ALU.add)
                elif mode == "a":
                    t2 = atmp[:, 0, :]
                    t3 = atmp[:, 1, :]
                    nc.scalar.activation(out=t2, in_=g_sb[:, k, 2 * C:3 * C],
                                         func=AF.Identity,
                                         scale=wts_sb[:, t, 3 * k + 1:3 * k + 2])
                    nc.scalar.activation(out=t3, in_=g_sb[:, k, 3 * C:4 * C],
                                         func=AF.Identity,
                                         scale=wts_sb[:, t, 3 * k + 2:3 * k + 3])
                    nc.vector.scalar_tensor_tensor(
                        out=av, in0=g_sb[:, k, C:2 * C],
                        scalar=wts_sb[:, t, 3 * k:3 * k + 1],
                        in1=g_sb[:, k, 0:C], op0=ALU.mult, op1=ALU.add)
                    nc.vector.tensor_tensor(out=av, in0=av, in1=t2, op=ALU.add)
                    nc.vector.tensor_tensor(out=av, in0=av, in1=t3, op=ALU.add)
                else:   # 'p': diags pre-built by issue_d3
                    pass

            sampT = smp.tile([128, KT, 128], BF16, tag="st")
            for q in range(5):   # groups of 4 kt-slots -> one evac
                n_in_g = 4 if q < 4 else 2
                ps = psS.tile([128, 4 * 128], FP32, tag="pss")
                for j in range(n_in_g):
                    kt = q * 4 + j
                    k = kt // 2
                    h = kt % 2
                    slot = ps[:, j * 128:(j + 1) * 128]
                    if MODES[k] == "p":
                        gk = g_sb[:, k, :]
                        nc.tensor.matmul(out=slot, lhsT=gk[:, h * 128:h * 128 + 128],
                                         rhs=ident_b[:],
                                         start=True, stop=False)
                        for s in range(3):
                            nc.tensor.matmul(
                                out=slot,
                                lhsT=gk[:, (s + 1) * C + h * 128:
                                        (s + 1) * C + h * 128 + 128],
                                rhs=d3[:, k, s, :],
                                start=False, stop=(s == 2))
                    else:
                        nc.tensor.matmul(
                            out=slot, lhsT=samp[:, kt * 128:(kt + 1) * 128],
                            rhs=ident_b[:], start=True, stop=True)
                ev = EVACS[q]
                dst = sampT[:, q * 4:q * 4 + n_in_g, :]
                src = ps[:, :n_in_g * 128]
                if ev == "A":
                    nc.scalar.copy(dst, src)
                elif ev == "D":
                    nc.vector.tensor_copy(dst, src)
                else:
                    nc.gpsimd.tensor_copy(dst, src)
            for ot in range(2):
                pso = psO.tile([128, 128], FP32, tag="po")
                for kt in range(KT):
                    nc.tensor.matmul(out=pso[:],
                                     lhsT=w_defT[:, kt, ot * 128:(ot + 1) * 128],
                                     rhs=sampT[:, kt, :],
                                     start=(kt == 0), stop=(kt == KT - 1))
                ob = oev.tile([128, 128], FP32, tag="ob")
                if OUT_EVAC == "A":
                    nc.scalar.copy(ob[:], pso[:])
                elif OUT_EVAC == "D":
                    nc.vector.tensor_copy(ob[:], pso[:])
                else:
                    nc.gpsimd.tensor_copy(ob[:], pso[:])
                nc.sync.dma_start(
                    out=out_d[ot * 128:(ot + 1) * 128, t * 128:(t + 1) * 128],
                    in_=ob[:])
    return nc


_CACHE = {}


def _get_nc():
    if "nc" not in _CACHE:
        nc = build_nc()
        if not nc.is_finalized():
            nc.finalize()
        _CACHE["nc"] = nc
    return _CACHE["nc"]


def kernel(**inputs):
    from concourse import bass_utils
    x = np.ascontiguousarray(inputs["x"], dtype=np.float32)          # [8,256,64,64]
    w_adj = np.ascontiguousarray(inputs["w_adj"], dtype=np.float32).reshape(OFFC, C)
    b_adj = np.ascontiguousarray(inputs["b_adj"], dtype=np.float32).reshape(OFFC, 1)
    w_off = np.ascontiguousarray(inputs["w_off"], dtype=np.float32).reshape(OFFC, K)
    b_off = np.ascontiguousarray(inputs["b_off"], dtype=np.float32).reshape(OFFC, 1)
    w_def = np.ascontiguousarray(inputs["w_def"], dtype=np.float32).reshape(C, C * K)

    nc = _get_nc()
    in_maps = []
    for n in range(N):
        in_maps.append({
            "x": np.ascontiguousarray(x[n].reshape(C, HW)),
            "w_adj": w_adj, "b_adj": b_adj,
            "w_off": w_off, "b_off": b_off,
            "w_def": w_def,
        })
    res = bass_utils.run_bass_kernel_spmd(nc, in_maps, core_ids=list(range(N)))
    outs = [res.results[n]["out"].reshape(C, H, W) for n in range(N)]
    return np.stack(outs, axis=0)


if __name__ == "__main__":
    nc = build_nc()
    print("build ok")
